# revision 3
# baseline (speedup 1.0000x reference)
"""ClusterAwareBatchNorm2d on 8 Trainium2 NeuronCores.

Strategy (batch-sharded, single kernel launch):
  - Each core owns 8 of the 64 samples (contiguous slab of x).
  - Pass 1: stream the core's x shard [8, 2, 128, 3136] through SBUF,
    computing per-(b,c) sum (DVE reduce) and sum-of-squares (ACT Square
    with accum_out) -> per-sample mean / unbiased var in [c, b] layout.
  - Tiny AllGather (16 KB/rank) of the per-sample [b, c] stats.
  - Every core redundantly runs FINCH first-partition clustering on-chip:
    gram matrix via PE, 1-NN via masked row-max + is_equal, connected
    components via 6 boolean matrix squarings (reachability closure),
    then cluster mean/var in matrix form (M @ stats), folded into a
    per-(b,c) affine A*x + B.
  - A per-core one-hot selection matrix (host input) picks the core's own
    8 rows of A/B; PE transposes them to [c, b] so pass 2 applies the
    affine with per-partition scale/bias on the ACT engine.
  - Pass 2: re-stream x, out = A*x + B, DMA out.
"""

import numpy as np
from contextlib import ExitStack

import concourse.bass as bass
import concourse.bacc as bacc
import concourse.tile as tile
import concourse.mybir as mybir
from concourse import bass_utils
from concourse.bass_interp import get_hw_module

F32 = mybir.dt.float32
AF = mybir.ActivationFunctionType
ALU = mybir.AluOpType
AX = mybir.AxisListType

B, C, H, W = 64, 256, 56, 56
HW = H * W                      # 3136
NCORES = 8
BL = B // NCORES                # 8 samples per core
CT = C // 128                   # 2 channel tiles
EPS = 1e-5
NEG = -1.0e30


def build_program(rate_: float):
    nc = bacc.Bacc(
        "TRN2",
        target_bir_lowering=False,
        debug=False,
        num_devices=NCORES,
    )

    x_d = nc.dram_tensor("x", [BL, CT, 128, HW], F32, kind="ExternalInput")
    vb_d = nc.dram_tensor("vb", [B, C], F32, kind="ExternalInput")
    mb_d = nc.dram_tensor("mb", [B, C], F32, kind="ExternalInput")
    wt_d = nc.dram_tensor("wt", [B, C], F32, kind="ExternalInput")
    bs_d = nc.dram_tensor("bs", [B, C], F32, kind="ExternalInput")
    sel_d = nc.dram_tensor("sel", [B, BL], F32, kind="ExternalInput")
    id_d = nc.dram_tensor("ident", [128, 128], F32, kind="ExternalInput")
    out_d = nc.dram_tensor("out", [BL, CT, 128, HW], F32, kind="ExternalOutput")

    with tile.TileContext(nc, num_cores=NCORES) as tc, ExitStack() as ctx:
        sb = ctx.enter_context(tc.tile_pool(name="sb", bufs=1))
        xin = ctx.enter_context(tc.tile_pool(name="xin", bufs=3))
        xin2 = ctx.enter_context(tc.tile_pool(name="xin2", bufs=3))
        yout = ctx.enter_context(tc.tile_pool(name="yout", bufs=3))
        ps = ctx.enter_context(tc.tile_pool(name="ps", bufs=2, space="PSUM"))
        ps1 = ctx.enter_context(tc.tile_pool(name="ps1", bufs=1, space="PSUM"))
        dram = ctx.enter_context(tc.tile_pool(name="dram", bufs=1, space="DRAM"))

        ident = sb.tile([128, 128], F32, tag="ident")
        nc.sync.dma_start(out=ident, in_=id_d[:, :])
        sel_sb = sb.tile([B, BL], F32, tag="sel")
        nc.sync.dma_start(out=sel_sb, in_=sel_d[:, :])
        vb_sb = sb.tile([B, C], F32, tag="vb")
        nc.sync.dma_start(out=vb_sb, in_=vb_d[:, :])
        mb_sb = sb.tile([B, C], F32, tag="mb")
        nc.sync.dma_start(out=mb_sb, in_=mb_d[:, :])
        wt_sb = sb.tile([B, C], F32, tag="wt")
        nc.sync.dma_start(out=wt_sb, in_=wt_d[:, :])
        bs_sb = sb.tile([B, C], F32, tag="bs")
        nc.sync.dma_start(out=bs_sb, in_=bs_d[:, :])

        # ---- pass 1: per-(b, c) sum and sum of squares --------------------
        s1_cb = [sb.tile([128, BL], F32, tag=f"s1_{t}", name=f"s1_{t}") for t in range(CT)]
        s2_cb = [sb.tile([128, BL], F32, tag=f"s2_{t}", name=f"s2_{t}") for t in range(CT)]
        sq_scr = sb.tile([128, HW], F32, tag="sq_scr")

        for b in range(BL):
            for t in range(CT):
                xt = xin.tile([128, HW], F32, tag="xt", name=f"xt_{b}_{t}")
                nc.sync.dma_start(out=xt, in_=x_d[b, t])
                nc.vector.reduce_sum(out=s1_cb[t][:, b : b + 1], in_=xt, axis=AX.X)
                nc.scalar.activation(
                    out=sq_scr,
                    in_=xt,
                    func=AF.Square,
                    accum_out=s2_cb[t][:, b : b + 1],
                )

        # finalize: mean + unbiased var, packed [128, 2, BL] per c-tile
        pack = []
        for t in range(CT):
            pk = sb.tile([128, 2, BL], F32, tag=f"pack_{t}", name=f"pack_{t}")
            nc.scalar.mul(out=pk[:, 0, :], in_=s1_cb[t], mul=1.0 / HW)
            musq = sb.tile([128, BL], F32, tag="musq", name=f"musq_{t}")
            nc.vector.tensor_mul(musq, pk[:, 0, :], pk[:, 0, :])
            s2n = sb.tile([128, BL], F32, tag="s2n", name=f"s2n_{t}")
            nc.scalar.mul(out=s2n, in_=s2_cb[t], mul=1.0 / (HW - 1))
            nc.vector.scalar_tensor_tensor(
                out=pk[:, 1, :],
                in0=musq,
                scalar=-HW / (HW - 1.0),
                in1=s2n,
                op0=ALU.mult,
                op1=ALU.add,
            )
            pack.append(pk)

        # ---- AllGather of the tiny per-sample stats -----------------------
        cc_in = dram.tile([CT, 2 * BL, 128], F32, name="cc_in")
        cc_out = dram.tile([NCORES, CT, 2 * BL, 128], F32, name="cc_out")

        for t in range(CT):
            pt = ps.tile([2 * BL, 128], F32, tag="pt", name=f"pt_{t}")
            nc.tensor.transpose(pt, pack[t].rearrange("p a b -> p (a b)"), ident)
            loc = sb.tile([2 * BL, 128], F32, tag="loc", name=f"loc_{t}")
            nc.scalar.copy(out=loc, in_=pt)
            nc.sync.dma_start(out=cc_in[t], in_=loc)

        nc.gpsimd.collective_compute(
            "AllGather",
            ALU.bypass,
            replica_groups=[list(range(NCORES))],
            ins=[cc_in.opt()],
            outs=[cc_out.opt()],
        )

        # cc_out layout: [rank, t, (m b_loc), p] with m=0 -> mean, m=1 -> var
        v = cc_out.rearrange("r t (m b) p -> m t r b p", m=2)
        mu_bc = sb.tile([B, CT, 128], F32, tag="mu_bc")   # [64, 256] as [64, 2, 128]
        sig_bc = sb.tile([B, CT, 128], F32, tag="sig_bc")
        for t in range(CT):
            nc.sync.dma_start(out=mu_bc[:, t, :], in_=v[0, t])
            nc.sync.dma_start(out=sig_bc[:, t, :], in_=v[1, t])
        mu_bc2 = mu_bc.rearrange("q t p -> q (t p)")      # [64, 256] views
        sig_bc2 = sig_bc.rearrange("q t p -> q (t p)")

        # ---- FINCH: 1-NN graph + connected-component closure --------------
        i64 = ident[:B, :B]
        mu_cb = []
        for t in range(CT):
            pt2 = ps.tile([128, B], F32, tag="pt", name=f"pt2_{t}")
            nc.tensor.transpose(pt2, mu_bc[:, t, :], i64)
            mcb = sb.tile([128, B], F32, tag=f"mucb_{t}", name=f"mucb_{t}")
            nc.scalar.copy(out=mcb, in_=pt2)
            mu_cb.append(mcb)

        g_ps = ps1.tile([B, B], F32, tag="g", name="g_ps")
        nc.tensor.matmul(g_ps, lhsT=mu_cb[0], rhs=mu_cb[0], start=True, stop=False)
        nc.tensor.matmul(g_ps, lhsT=mu_cb[1], rhs=mu_cb[1], start=False, stop=True)
        g_sb = sb.tile([B, B], F32, tag="g_sb")
        nc.scalar.copy(out=g_sb, in_=g_ps)

        dtmp = sb.tile([B, B], F32, tag="dtmp")
        nc.vector.tensor_mul(dtmp, g_sb, i64)
        dg = sb.tile([B, 1], F32, tag="dg")
        nc.vector.reduce_sum(out=dg, in_=dtmp, axis=AX.X)
        rdg0 = sb.tile([B, 1], F32, tag="rdg0")
        nc.vector.reciprocal(rdg0, dg)
        rdg = sb.tile([B, 1], F32, tag="rdg")
        nc.scalar.sqrt(rdg, rdg0)                         # 1/||mu_j||

        d_sb = sb.tile([B, B], F32, tag="d_sb")           # rows j scaled by rdg[j]
        nc.vector.tensor_scalar_mul(d_sb, g_sb, rdg)
        c_ps = ps.tile([B, B], F32, tag="pg", name="c_ps")
        nc.tensor.transpose(c_ps, d_sb, i64)              # C[i,j] = G[i,j]/||mu_j||
        c_m = sb.tile([B, B], F32, tag="c_m")
        nc.vector.scalar_tensor_tensor(
            out=c_m, in0=i64, scalar=NEG, in1=c_ps, op0=ALU.mult, op1=ALU.add
        )
        mx = sb.tile([B, 1], F32, tag="mx")
        nc.vector.reduce_max(out=mx, in_=c_m, axis=AX.X)
        p_sb = sb.tile([B, B], F32, tag="p_sb")           # one-hot nearest neighbor
        nc.vector.tensor_scalar(out=p_sb, in0=c_m, scalar1=mx, scalar2=None, op0=ALU.is_equal)

        pt_ps = ps.tile([B, B], F32, tag="pg", name="pt_ps")
        nc.tensor.transpose(pt_ps, p_sb, i64)
        pt_sb = sb.tile([B, B], F32, tag="pt_sb")
        nc.scalar.copy(out=pt_sb, in_=pt_ps)
        ppt_ps = ps.tile([B, B], F32, tag="pg", name="ppt_ps")
        nc.tensor.matmul(ppt_ps, lhsT=pt_sb, rhs=pt_sb)   # P @ P.T

        acc1 = sb.tile([B, B], F32, tag="acc1")
        nc.vector.tensor_add(acc1, p_sb, pt_sb)
        acc2 = sb.tile([B, B], F32, tag="acc2")
        nc.vector.tensor_add(acc2, acc1, i64)
        acc3 = sb.tile([B, B], F32, tag="acc3")
        nc.vector.scalar_tensor_tensor(
            out=acc3, in0=ppt_ps, scalar=1.0, in1=acc2, op0=ALU.mult, op1=ALU.add
        )
        r_cur = sb.tile([B, B], F32, tag="r0", name="r0")
        nc.vector.tensor_scalar(out=r_cur, in0=acc3, scalar1=0.5, scalar2=None, op0=ALU.is_ge)

        for it in range(6):                               # R^(2^6) covers paths <= 64
            s_ps = ps.tile([B, B], F32, tag="pg", name=f"s_ps{it}")
            nc.tensor.matmul(s_ps, lhsT=r_cur, rhs=r_cur)
            r_nxt = sb.tile([B, B], F32, tag=f"r{(it % 2) + 1}", name=f"r{it + 1}")
            nc.vector.tensor_scalar(out=r_nxt, in0=s_ps, scalar1=0.5, scalar2=None, op0=ALU.is_ge)
            r_cur = r_nxt

        # ---- cluster stats in matrix form ---------------------------------
        rowN = sb.tile([B, 1], F32, tag="rowN")
        nc.vector.reduce_sum(out=rowN, in_=r_cur, axis=AX.X)
        dE = sb.tile([B, 1], F32, tag="dE")
        nc.vector.tensor_scalar(out=dE, in0=rowN, scalar1=float(EPS), scalar2=None, op0=ALU.add)
        rinv = sb.tile([B, 1], F32, tag="rinv")
        nc.vector.reciprocal(rinv, dE)

        s1_ps = ps1.tile([B, C], F32, tag="s1p", name="s1_ps")
        nc.tensor.matmul(s1_ps, lhsT=r_cur, rhs=mu_bc2)
        mu_g = sb.tile([B, C], F32, tag="mu_g")
        nc.vector.tensor_scalar_mul(mu_g, s1_ps, rinv)

        ss_ps = ps.tile([B, C], F32, tag="ps_big", name="ss_ps")
        nc.tensor.matmul(ss_ps, lhsT=r_cur, rhs=sig_bc2)
        sig_w = sb.tile([B, C], F32, tag="sig_w")
        nc.vector.tensor_scalar_mul(sig_w, ss_ps, rinv)

        mu2 = sb.tile([B, C], F32, tag="mu2")
        nc.vector.tensor_mul(mu2, mu_bc2, mu_bc2)
        s2_ps = ps.tile([B, C], F32, tag="ps_big", name="s2_ps")
        nc.tensor.matmul(s2_ps, lhsT=r_cur, rhs=mu2)

        tA = sb.tile([B, C], F32, tag="tA")
        nc.vector.tensor_scalar_mul(tA, mu_g, rowN)
        tB = sb.tile([B, C], F32, tag="tB")
        nc.vector.scalar_tensor_tensor(
            out=tB, in0=s1_ps, scalar=2.0, in1=tA, op0=ALU.mult, op1=ALU.subtract
        )
        tC = sb.tile([B, C], F32, tag="tC")
        nc.vector.tensor_mul(tC, mu_g, tB)
        d2 = sb.tile([B, C], F32, tag="d2")
        nc.vector.tensor_sub(d2, s2_ps, tC)
        sig_g = sb.tile([B, C], F32, tag="sig_g")
        nc.vector.scalar_tensor_tensor(
            out=sig_g, in0=d2, scalar=rinv, in1=sig_w, op0=ALU.mult, op1=ALU.add
        )

        # fused affine: out = A * x + Bc
        vV = sb.tile([B, C], F32, tag="vV")
        nc.vector.scalar_tensor_tensor(
            out=vV, in0=sig_g, scalar=float(rate_), in1=vb_sb, op0=ALU.mult, op1=ALU.add
        )
        vr = sb.tile([B, C], F32, tag="vr")
        nc.vector.reciprocal(vr, vV)
        rq = sb.tile([B, C], F32, tag="rq")
        nc.scalar.sqrt(rq, vr)                            # rsqrt(V)
        a_t = sb.tile([B, C], F32, tag="a_t")
        nc.vector.tensor_mul(a_t, rq, wt_sb)
        t4 = sb.tile([B, C], F32, tag="t4")
        nc.vector.scalar_tensor_tensor(
            out=t4, in0=mu_g, scalar=float(rate_), in1=mb_sb, op0=ALU.mult, op1=ALU.add
        )
        t5 = sb.tile([B, C], F32, tag="t5")
        nc.vector.tensor_mul(t5, t4, a_t)
        b_t = sb.tile([B, C], F32, tag="b_t")
        nc.vector.tensor_sub(b_t, bs_sb, t5)

        # select this core's 8 rows, transpose to [c, b_loc]
        asel_ps = ps.tile([BL, C], F32, tag="ps_big", name="asel_ps")
        nc.tensor.matmul(asel_ps, lhsT=sel_sb, rhs=a_t)
        asel = sb.tile([BL, C], F32, tag="asel")
        nc.scalar.copy(out=asel, in_=asel_ps)
        bsel_ps = ps.tile([BL, C], F32, tag="ps_big", name="bsel_ps")
        nc.tensor.matmul(bsel_ps, lhsT=sel_sb, rhs=b_t)
        bsel = sb.tile([BL, C], F32, tag="bsel")
        nc.scalar.copy(out=bsel, in_=bsel_ps)

        a_own, b_own = [], []
        for t in range(CT):
            ta_ps = ps.tile([128, BL], F32, tag="pt", name=f"ta_{t}")
            nc.tensor.transpose(ta_ps, asel[:, t * 128 : (t + 1) * 128], ident[:BL, :BL])
            ao = sb.tile([128, BL], F32, tag=f"aown_{t}", name=f"aown_{t}")
            nc.scalar.copy(out=ao, in_=ta_ps)
            a_own.append(ao)
            tb_ps = ps.tile([128, BL], F32, tag="pt", name=f"tb_{t}")
            nc.tensor.transpose(tb_ps, bsel[:, t * 128 : (t + 1) * 128], ident[:BL, :BL])
            bo = sb.tile([128, BL], F32, tag=f"bown_{t}", name=f"bown_{t}")
            nc.scalar.copy(out=bo, in_=tb_ps)
            b_own.append(bo)

        # ---- pass 2: fused normalize --------------------------------------
        for b in range(BL):
            for t in range(CT):
                xt2 = xin2.tile([128, HW], F32, tag="xt2", name=f"xt2_{b}_{t}")
                nc.sync.dma_start(out=xt2, in_=x_d[b, t])
                yt = yout.tile([128, HW], F32, tag="yt", name=f"yt_{b}_{t}")
                nc.scalar.activation(
                    out=yt,
                    in_=xt2,
                    func=AF.Identity,
                    bias=b_own[t][:, b : b + 1],
                    scale=a_own[t][:, b : b + 1],
                )
                nc.sync.dma_start(out=out_d[b, t], in_=yt)

    nc.compile()
    nc.m = get_hw_module(nc.m)
    return nc


_CACHE: dict = {}


def _prepare(x, running_mean, running_var, weight, bias, source_rate):
    x = np.ascontiguousarray(np.asarray(x, dtype=np.float32))
    rm = np.asarray(running_mean, np.float32)
    rv = np.asarray(running_var, np.float32)
    wt = np.asarray(weight, np.float32)
    bs = np.asarray(bias, np.float32)
    sr = np.float32(min(max(float(np.asarray(source_rate)), 0.0), 1.0))
    rate_ = float(np.float32(1.0) - sr)

    vb = (sr * rv + np.float32(EPS)).astype(np.float32)
    mb = (sr * rm).astype(np.float32)
    vb_bc = np.ascontiguousarray(np.broadcast_to(vb, (B, C)))
    mb_bc = np.ascontiguousarray(np.broadcast_to(mb, (B, C)))
    wt_bc = np.ascontiguousarray(np.broadcast_to(wt, (B, C)))
    bs_bc = np.ascontiguousarray(np.broadcast_to(bs, (B, C)))
    ident = np.eye(128, dtype=np.float32)

    in_maps = []
    for k in range(NCORES):
        sel = np.zeros((B, BL), np.float32)
        sel[k * BL + np.arange(BL), np.arange(BL)] = 1.0
        in_maps.append(
            {
                "x": x[k * BL : (k + 1) * BL].reshape(BL, CT, 128, HW),
                "vb": vb_bc,
                "mb": mb_bc,
                "wt": wt_bc,
                "bs": bs_bc,
                "sel": sel,
                "ident": ident,
            }
        )
    return rate_, in_maps


def run(inputs: dict, trace: bool = False):
    rate_, in_maps = _prepare(**inputs)
    if rate_ not in _CACHE:
        _CACHE[rate_] = build_program(rate_)
    nc = _CACHE[rate_]
    res = bass_utils.run_bass_kernel_spmd(
        nc, in_maps, core_ids=list(range(NCORES)), trace=trace
    )
    outs = [np.asarray(r["out"]).reshape(BL, C, H, W) for r in res.results]
    return np.concatenate(outs, axis=0), res


def kernel(**inputs) -> np.ndarray:
    out, _ = run(inputs)
    return out


# revision 4
# speedup vs baseline: 1.6922x; 1.6922x over previous
"""ClusterAwareBatchNorm2d on 8 Trainium2 NeuronCores.

Strategy (batch-sharded, single kernel launch):
  - Each core owns 8 of the 64 samples (contiguous slab of x).
  - Pass 1: stream the core's x shard [8, 2, 128, 3136] through SBUF,
    computing per-(b,c) sum (DVE reduce) and sum-of-squares (ACT Square
    with accum_out) -> per-sample mean / unbiased var in [c, b] layout.
    11 of the 16 tiles stay resident in SBUF for pass 2.
  - Tiny AllGather (16 KB/rank) of the per-sample [b, c] stats.
  - Every core redundantly runs FINCH first-partition clustering on-chip:
    gram matrix via PE, 1-NN via masked row-max + is_equal, connected
    components via 6 boolean matrix squarings (reachability closure),
    then cluster mean/var in matrix form (M @ stats), folded into a
    per-(b,c) affine A*x + B.
  - A per-core one-hot selection matrix (host input) picks the core's own
    8 rows of A/B; PE transposes them to [c, b] per-partition scale/bias.
  - Pass 2: normalize in place on the ACT engine (resident tiles need no
    reload; 5 streamed tiles are prefetched during the collective wait),
    stores issued from the ACT engine's own HWDGE queue so they never
    head-of-line-block the SP load queue.
"""

import numpy as np
from contextlib import ExitStack

import concourse.bass as bass
import concourse.bacc as bacc
import concourse.tile as tile
import concourse.mybir as mybir
from concourse import bass_utils
from concourse.bass_interp import get_hw_module

F32 = mybir.dt.float32
AF = mybir.ActivationFunctionType
ALU = mybir.AluOpType
AX = mybir.AxisListType

B, C, H, W = 64, 256, 56, 56
HW = H * W                      # 3136
NCORES = 8
BL = B // NCORES                # 8 samples per core
CT = C // 128                   # 2 channel tiles
NTILES = BL * CT                # 16 x-tiles of [128, HW] per core
NRES = 11                       # tiles kept resident in SBUF across passes
EPS = 1e-5
NEG = -1.0e30


def build_program(rate_: float):
    nc = bacc.Bacc(
        "TRN2",
        target_bir_lowering=False,
        debug=False,
        num_devices=NCORES,
    )

    x_d = nc.dram_tensor("x", [BL, CT, 128, HW], F32, kind="ExternalInput")
    vb_d = nc.dram_tensor("vb", [B, C], F32, kind="ExternalInput")
    mb_d = nc.dram_tensor("mb", [B, C], F32, kind="ExternalInput")
    wt_d = nc.dram_tensor("wt", [B, C], F32, kind="ExternalInput")
    bs_d = nc.dram_tensor("bs", [B, C], F32, kind="ExternalInput")
    sel_d = nc.dram_tensor("sel", [B, BL], F32, kind="ExternalInput")
    id_d = nc.dram_tensor("ident", [128, 128], F32, kind="ExternalInput")
    out_d = nc.dram_tensor("out", [BL, CT, 128, HW], F32, kind="ExternalOutput")

    idx_all = list(range(NTILES))               # idx = b*CT + t
    idx_stream = idx_all[: NTILES - NRES]
    idx_res = idx_all[NTILES - NRES :]

    with tile.TileContext(nc, num_cores=NCORES) as tc, ExitStack() as ctx:
        sb = ctx.enter_context(tc.tile_pool(name="sb", bufs=1))
        res = ctx.enter_context(tc.tile_pool(name="res", bufs=NRES))
        xs = ctx.enter_context(tc.tile_pool(name="xs", bufs=2))
        ps = ctx.enter_context(tc.tile_pool(name="ps", bufs=2, space="PSUM"))
        ps1 = ctx.enter_context(tc.tile_pool(name="ps1", bufs=1, space="PSUM"))
        dram = ctx.enter_context(tc.tile_pool(name="dram", bufs=1, space="DRAM"))

        # small constants via SWDGE (keeps the SP HWDGE queue free for x)
        ident = sb.tile([128, 128], F32, tag="ident")
        nc.gpsimd.dma_start(out=ident, in_=id_d[:, :])
        sel_sb = sb.tile([B, BL], F32, tag="sel")
        nc.gpsimd.dma_start(out=sel_sb, in_=sel_d[:, :])
        vb_sb = sb.tile([B, C], F32, tag="vb")
        nc.gpsimd.dma_start(out=vb_sb, in_=vb_d[:, :])
        mb_sb = sb.tile([B, C], F32, tag="mb")
        nc.gpsimd.dma_start(out=mb_sb, in_=mb_d[:, :])
        wt_sb = sb.tile([B, C], F32, tag="wt")
        nc.gpsimd.dma_start(out=wt_sb, in_=wt_d[:, :])
        bs_sb = sb.tile([B, C], F32, tag="bs")
        nc.gpsimd.dma_start(out=bs_sb, in_=bs_d[:, :])

        # ---- pass 1: per-(b, c) sum and sum of squares --------------------
        s1_cb = [sb.tile([128, BL], F32, tag=f"s1_{t}", name=f"s1_{t}") for t in range(CT)]
        s2_cb = [sb.tile([128, BL], F32, tag=f"s2_{t}", name=f"s2_{t}") for t in range(CT)]
        sq_scr = sb.tile([128, HW], F32, tag="sq_scr")

        xtile = {}
        for i in idx_stream + idx_res:
            b, t = divmod(i, CT)
            pool, tag = (res, "res") if i in idx_res else (xs, "xs")
            xt = pool.tile([128, HW], F32, tag=tag, name=f"xt_{b}_{t}")
            xtile[i] = xt
            nc.sync.dma_start(out=xt, in_=x_d[b, t])
            nc.vector.reduce_sum(out=s1_cb[t][:, b : b + 1], in_=xt, axis=AX.X)
            nc.scalar.activation(
                out=sq_scr,
                in_=xt,
                func=AF.Square,
                accum_out=s2_cb[t][:, b : b + 1],
            )

        # finalize: mean + unbiased var, packed [128, 2, BL] per c-tile
        pack = []
        for t in range(CT):
            pk = sb.tile([128, 2, BL], F32, tag=f"pack_{t}", name=f"pack_{t}")
            nc.scalar.mul(out=pk[:, 0, :], in_=s1_cb[t], mul=1.0 / HW)
            musq = sb.tile([128, BL], F32, tag="musq", name=f"musq_{t}")
            nc.vector.tensor_mul(musq, pk[:, 0, :], pk[:, 0, :])
            s2n = sb.tile([128, BL], F32, tag="s2n", name=f"s2n_{t}")
            nc.scalar.mul(out=s2n, in_=s2_cb[t], mul=1.0 / (HW - 1))
            nc.vector.scalar_tensor_tensor(
                out=pk[:, 1, :],
                in0=musq,
                scalar=-HW / (HW - 1.0),
                in1=s2n,
                op0=ALU.mult,
                op1=ALU.add,
            )
            pack.append(pk)

        # ---- AllGather of the tiny per-sample stats -----------------------
        cc_in = dram.tile([CT, 2 * BL, 128], F32, name="cc_in")
        cc_out = dram.tile([NCORES, CT, 2 * BL, 128], F32, name="cc_out")

        for t in range(CT):
            pt = ps.tile([2 * BL, 128], F32, tag="pt", name=f"pt_{t}")
            nc.tensor.transpose(pt, pack[t].rearrange("p a b -> p (a b)"), ident)
            loc = sb.tile([2 * BL, 128], F32, tag="loc", name=f"loc_{t}")
            nc.scalar.copy(out=loc, in_=pt)
            nc.gpsimd.dma_start(out=cc_in[t], in_=loc)

        nc.gpsimd.collective_compute(
            "AllGather",
            ALU.bypass,
            replica_groups=[list(range(NCORES))],
            ins=[cc_in.opt()],
            outs=[cc_out.opt()],
        )

        # cc_out layout: [rank, t, (m b_loc), p] with m=0 -> mean, m=1 -> var
        v = cc_out.rearrange("r t (m b) p -> m t r b p", m=2)
        mu_bc = sb.tile([B, CT, 128], F32, tag="mu_bc")   # [64, 256] as [64, 2, 128]
        sig_bc = sb.tile([B, CT, 128], F32, tag="sig_bc")
        for t in range(CT):
            nc.gpsimd.dma_start(out=mu_bc[:, t, :], in_=v[0, t])
            nc.gpsimd.dma_start(out=sig_bc[:, t, :], in_=v[1, t])
        mu_bc2 = mu_bc.rearrange("q t p -> q (t p)")      # [64, 256] views
        sig_bc2 = sig_bc.rearrange("q t p -> q (t p)")

        # ---- FINCH: 1-NN graph + connected-component closure --------------
        i64 = ident[:B, :B]
        mu_cb = []
        for t in range(CT):
            pt2 = ps.tile([128, B], F32, tag="pt", name=f"pt2_{t}")
            nc.tensor.transpose(pt2, mu_bc[:, t, :], i64)
            mcb = sb.tile([128, B], F32, tag=f"mucb_{t}", name=f"mucb_{t}")
            nc.scalar.copy(out=mcb, in_=pt2)
            mu_cb.append(mcb)

        g_ps = ps1.tile([B, B], F32, tag="g", name="g_ps")
        nc.tensor.matmul(g_ps, lhsT=mu_cb[0], rhs=mu_cb[0], start=True, stop=False)
        nc.tensor.matmul(g_ps, lhsT=mu_cb[1], rhs=mu_cb[1], start=False, stop=True)

        dtmp = sb.tile([B, B], F32, tag="dtmp")
        nc.vector.tensor_mul(dtmp, g_ps, i64)
        dg = sb.tile([B, 1], F32, tag="dg")
        nc.vector.reduce_sum(out=dg, in_=dtmp, axis=AX.X)
        rdg0 = sb.tile([B, 1], F32, tag="rdg0")
        nc.vector.reciprocal(rdg0, dg)
        rdg = sb.tile([B, 1], F32, tag="rdg")
        nc.scalar.sqrt(rdg, rdg0)                         # 1/||mu_j||

        d_sb = sb.tile([B, B], F32, tag="d_sb")           # rows j scaled by rdg[j]
        nc.vector.tensor_scalar_mul(d_sb, g_ps, rdg)
        c_ps = ps.tile([B, B], F32, tag="pg", name="c_ps")
        nc.tensor.transpose(c_ps, d_sb, i64)              # C[i,j] = G[i,j]/||mu_j||
        c_m = sb.tile([B, B], F32, tag="c_m")
        nc.vector.scalar_tensor_tensor(
            out=c_m, in0=i64, scalar=NEG, in1=c_ps, op0=ALU.mult, op1=ALU.add
        )
        mx = sb.tile([B, 1], F32, tag="mx")
        nc.vector.reduce_max(out=mx, in_=c_m, axis=AX.X)
        p_sb = sb.tile([B, B], F32, tag="p_sb")           # one-hot nearest neighbor
        nc.vector.tensor_scalar(out=p_sb, in0=c_m, scalar1=mx, scalar2=None, op0=ALU.is_equal)

        pt_ps = ps.tile([B, B], F32, tag="pg", name="pt_ps")
        nc.tensor.transpose(pt_ps, p_sb, i64)
        pt_sb = sb.tile([B, B], F32, tag="pt_sb")
        nc.scalar.copy(out=pt_sb, in_=pt_ps)
        ppt_ps = ps.tile([B, B], F32, tag="pg", name="ppt_ps")
        nc.tensor.matmul(ppt_ps, lhsT=pt_sb, rhs=pt_sb)   # P @ P.T  (diag == 1)

        acc1 = sb.tile([B, B], F32, tag="acc1")
        nc.vector.tensor_add(acc1, p_sb, pt_sb)
        acc3 = sb.tile([B, B], F32, tag="acc3")
        nc.vector.scalar_tensor_tensor(
            out=acc3, in0=ppt_ps, scalar=1.0, in1=acc1, op0=ALU.mult, op1=ALU.add
        )
        r_cur = sb.tile([B, B], F32, tag="r0", name="r0")
        nc.vector.tensor_scalar(out=r_cur, in0=acc3, scalar1=0.5, scalar2=None, op0=ALU.is_ge)

        for it in range(6):                               # R^(2^6) covers paths <= 64
            s_ps = ps.tile([B, B], F32, tag="pg", name=f"s_ps{it}")
            nc.tensor.matmul(s_ps, lhsT=r_cur, rhs=r_cur)
            r_nxt = sb.tile([B, B], F32, tag=f"r{(it % 2) + 1}", name=f"r{it + 1}")
            nc.vector.tensor_scalar(out=r_nxt, in0=s_ps, scalar1=0.5, scalar2=None, op0=ALU.is_ge)
            r_cur = r_nxt

        # ---- cluster stats in matrix form ---------------------------------
        rowN = sb.tile([B, 1], F32, tag="rowN")
        nc.vector.reduce_sum(out=rowN, in_=r_cur, axis=AX.X)
        dE = sb.tile([B, 1], F32, tag="dE")
        nc.vector.tensor_scalar(out=dE, in0=rowN, scalar1=float(EPS), scalar2=None, op0=ALU.add)
        rinv = sb.tile([B, 1], F32, tag="rinv")
        nc.vector.reciprocal(rinv, dE)

        s1_ps = ps1.tile([B, C], F32, tag="s1p", name="s1_ps")
        nc.tensor.matmul(s1_ps, lhsT=r_cur, rhs=mu_bc2)
        mu_g = sb.tile([B, C], F32, tag="mu_g")
        nc.vector.tensor_scalar_mul(mu_g, s1_ps, rinv)

        # sig_g = (M @ (sigma2 + mu^2)) * rinv - mu_g^2   (EPS-order exact
        # to ~1e-9; the rowN/(rowN+EPS) factor on mu_g^2 is dropped)
        mu2 = sb.tile([B, C], F32, tag="mu2")
        nc.vector.tensor_mul(mu2, mu_bc2, mu_bc2)
        smu = sb.tile([B, C], F32, tag="smu")
        nc.vector.tensor_add(smu, mu2, sig_bc2)
        ss_ps = ps.tile([B, C], F32, tag="ssp", name="ss_ps")
        nc.tensor.matmul(ss_ps, lhsT=r_cur, rhs=smu)
        mgsq = sb.tile([B, C], F32, tag="mgsq")
        nc.vector.tensor_mul(mgsq, mu_g, mu_g)
        sig_g = sb.tile([B, C], F32, tag="sig_g")
        nc.vector.scalar_tensor_tensor(
            out=sig_g, in0=ss_ps, scalar=rinv, in1=mgsq, op0=ALU.mult, op1=ALU.subtract
        )

        # fused affine: out = A * x + Bc
        vV = sb.tile([B, C], F32, tag="vV")
        nc.vector.scalar_tensor_tensor(
            out=vV, in0=sig_g, scalar=float(rate_), in1=vb_sb, op0=ALU.mult, op1=ALU.add
        )
        vr = sb.tile([B, C], F32, tag="vr")
        nc.vector.reciprocal(vr, vV)
        rq = sb.tile([B, C], F32, tag="rq")
        nc.scalar.sqrt(rq, vr)                            # rsqrt(V)
        a_t = sb.tile([B, C], F32, tag="a_t")
        nc.vector.tensor_mul(a_t, rq, wt_sb)
        t4 = sb.tile([B, C], F32, tag="t4")
        nc.vector.scalar_tensor_tensor(
            out=t4, in0=mu_g, scalar=float(rate_), in1=mb_sb, op0=ALU.mult, op1=ALU.add
        )
        t5 = sb.tile([B, C], F32, tag="t5")
        nc.vector.tensor_mul(t5, t4, a_t)
        b_t = sb.tile([B, C], F32, tag="b_t")
        nc.vector.tensor_sub(b_t, bs_sb, t5)

        # select this core's 8 rows, transpose to [c, b_loc]
        asel_ps = ps.tile([BL, C], F32, tag="ssp", name="asel_ps")
        nc.tensor.matmul(asel_ps, lhsT=sel_sb, rhs=a_t)
        asel = sb.tile([BL, C], F32, tag="asel")
        nc.scalar.copy(out=asel, in_=asel_ps)
        bsel_ps = ps.tile([BL, C], F32, tag="ssp", name="bsel_ps")
        nc.tensor.matmul(bsel_ps, lhsT=sel_sb, rhs=b_t)
        bsel = sb.tile([BL, C], F32, tag="bsel")
        nc.scalar.copy(out=bsel, in_=bsel_ps)

        a_own, b_own = [], []
        for t in range(CT):
            ta_ps = ps.tile([128, BL], F32, tag="pt", name=f"ta_{t}")
            nc.tensor.transpose(ta_ps, asel[:, t * 128 : (t + 1) * 128], ident[:BL, :BL])
            ao = sb.tile([128, BL], F32, tag=f"aown_{t}", name=f"aown_{t}")
            nc.scalar.copy(out=ao, in_=ta_ps)
            a_own.append(ao)
            tb_ps = ps.tile([128, BL], F32, tag="pt", name=f"tb_{t}")
            nc.tensor.transpose(tb_ps, bsel[:, t * 128 : (t + 1) * 128], ident[:BL, :BL])
            bo = sb.tile([128, BL], F32, tag=f"bown_{t}", name=f"bown_{t}")
            nc.scalar.copy(out=bo, in_=tb_ps)
            b_own.append(bo)

        # ---- pass 2: fused normalize, in place, stores on the ACT queue ---
        for i in idx_res:
            b, t = divmod(i, CT)
            xt = xtile[i]
            nc.scalar.activation(
                out=xt,
                in_=xt,
                func=AF.Identity,
                bias=b_own[t][:, b : b + 1],
                scale=a_own[t][:, b : b + 1],
            )
            nc.scalar.dma_start(out=out_d[b, t], in_=xt)
        for i in idx_stream:
            b, t = divmod(i, CT)
            xt2 = xs.tile([128, HW], F32, tag="xs", name=f"xt2_{b}_{t}")
            nc.sync.dma_start(out=xt2, in_=x_d[b, t])
            nc.scalar.activation(
                out=xt2,
                in_=xt2,
                func=AF.Identity,
                bias=b_own[t][:, b : b + 1],
                scale=a_own[t][:, b : b + 1],
            )
            nc.scalar.dma_start(out=out_d[b, t], in_=xt2)

    nc.compile()
    nc.m = get_hw_module(nc.m)
    return nc


_CACHE: dict = {}


def _prepare(x, running_mean, running_var, weight, bias, source_rate):
    x = np.ascontiguousarray(np.asarray(x, dtype=np.float32))
    rm = np.asarray(running_mean, np.float32)
    rv = np.asarray(running_var, np.float32)
    wt = np.asarray(weight, np.float32)
    bs = np.asarray(bias, np.float32)
    sr = np.float32(min(max(float(np.asarray(source_rate)), 0.0), 1.0))
    rate_ = float(np.float32(1.0) - sr)

    vb = (sr * rv + np.float32(EPS)).astype(np.float32)
    mb = (sr * rm).astype(np.float32)
    vb_bc = np.ascontiguousarray(np.broadcast_to(vb, (B, C)))
    mb_bc = np.ascontiguousarray(np.broadcast_to(mb, (B, C)))
    wt_bc = np.ascontiguousarray(np.broadcast_to(wt, (B, C)))
    bs_bc = np.ascontiguousarray(np.broadcast_to(bs, (B, C)))
    ident = np.eye(128, dtype=np.float32)

    in_maps = []
    for k in range(NCORES):
        sel = np.zeros((B, BL), np.float32)
        sel[k * BL + np.arange(BL), np.arange(BL)] = 1.0
        in_maps.append(
            {
                "x": x[k * BL : (k + 1) * BL].reshape(BL, CT, 128, HW),
                "vb": vb_bc,
                "mb": mb_bc,
                "wt": wt_bc,
                "bs": bs_bc,
                "sel": sel,
                "ident": ident,
            }
        )
    return rate_, in_maps


def run(inputs: dict, trace: bool = False, **kw):
    rate_, in_maps = _prepare(**inputs)
    if rate_ not in _CACHE:
        _CACHE[rate_] = build_program(rate_)
    nc = _CACHE[rate_]
    res = bass_utils.run_bass_kernel_spmd(
        nc, in_maps, core_ids=list(range(NCORES)), trace=trace, **kw
    )
    outs = [np.asarray(r["out"]).reshape(BL, C, H, W) for r in res.results]
    return np.concatenate(outs, axis=0), res


def kernel(**inputs) -> np.ndarray:
    out, _ = run(inputs)
    return out
